# revision 1
# baseline (speedup 1.0000x reference)
"""Trainium2 Bass kernel for BasicConvClassifierWithSubject.

Pure data parallel over 8 cores (128 items/core). Per item the network is a
chain of PE matmuls (convs as 3 shifted matmuls accumulating in PSUM, BN
folded into weights), gelu on ScalarE, residuals as identity/skip matmuls or
DVE adds. Spatial attention + per-subject 1x1 conv fuse on the host into one
per-item [271,128] stationary matrix (moe routing done as a host gather).

Items are processed in PAIRS sharing one 2-bank PSUM tile (item a at column 0,
item b at column 512), so each activation covers both items in a single
strided ScalarE instruction. Two pairs interleave (group of 4) so the PE
always has an independent pair's matmuls while ScalarE drains the other.
Early stages run f32r (tf32-like), the 256-channel tail runs bf16.
"""

import os
import numpy as np
import ml_dtypes

import concourse.bass as bass
import concourse.tile as tile
from concourse import bacc, mybir
from concourse.bass_utils import run_bass_kernel_spmd

f32 = mybir.dt.float32
f32r = mybir.dt.float32r
bf16 = mybir.dt.bfloat16
AF = mybir.ActivationFunctionType
AX = mybir.AxisListType

B, C, T = 1024, 271, 281
H, H2, E, NCLS, NSUBJ = 128, 256, 16, 1854, 4
EPS = 1e-5
TP = 284          # padded time length per chunk (zeros at cols 0, 282, 283)
N = 282           # matmul moving size (even)
KC = 3            # K chunks for the fused front matmul (271 -> 128+128+15)
PAR = 8           # items in flight (2 interleaved pairs x 2 groups deep)
NPP = PAR // 2    # pair slots

_CACHE = {}


def _stage_dtypes():
    mode = os.environ.get("KBF16", "mid")
    late = {"b2c2", "b3c1", "b3c2"}
    mid = late | {"b2c1", "b2sk"}
    allm = mid | {"front", "b1c1", "b1c2"}
    sel = {"none": set(), "late": late, "mid": mid, "all": allm}[mode]
    return {s: (bf16 if s in sel else f32r)
            for s in ("front", "b1c1", "b1c2", "b2c1", "b2sk", "b2c2", "b3c1", "b3c2")}


def _build(n_items, n_cores, sdt):
    assert n_items % 4 == 0
    nc = bacc.Bacc("TRN2", target_bir_lowering=False, debug=False,
                   num_devices=n_cores)

    xp_dt = sdt["front"]
    h0_dt = sdt["b1c1"]     # also read by b1c2 ident residual
    y1_dt = sdt["b1c2"]
    h1_dt = sdt["b2c1"]     # also read by b2 skip matmul
    y2_dt = sdt["b2c2"]
    h2_dt = sdt["b3c1"]     # also read by b3c2 DVE residual add
    y3_dt = sdt["b3c2"]

    def din(name, shape, dt=f32r):
        return nc.dram_tensor(name, shape, dt, kind="ExternalInput").ap()

    X = din("X", [n_items, C, T], xp_dt)
    Mg = din("Mg", [n_items, 128, KC * 128], sdt["front"])
    D = din("D", [128, n_items], f32)
    cbias = din("cbias", [128, 10], f32)
    w_b1c1 = din("w_b1c1", [128, 3 * 128], sdt["b1c1"])
    w_b1c2 = din("w_b1c2", [128, 3 * 128], sdt["b1c2"])
    w_b2c1 = din("w_b2c1", [128, 6 * 128], sdt["b2c1"])
    w_b2sk = din("w_b2sk", [128, 2 * 128], sdt["b2sk"])
    w_b2c2 = din("w_b2c2", [128, 12 * 128], sdt["b2c2"])
    w_b3c1 = din("w_b3c1", [128, 12 * 128], sdt["b3c1"])
    w_b3c2 = din("w_b3c2", [128, 12 * 128], sdt["b3c2"])
    ident_r = din("ident_r", [128, 128], sdt["b1c2"])
    w1 = din("w1", [128, 2 * 128])
    w1x = din("w1x", [128, 128])
    rhsx = din("rhsx", [128, n_items])
    w2t = din("w2t", [128, NCLS])
    b2row = din("b2row", [1, NCLS])
    ones1 = din("ones1", [1, n_items])
    out = nc.dram_tensor("out", [n_items, NCLS], f32, kind="ExternalOutput").ap()

    with tile.TileContext(nc) as tc:
        wpool = tc.alloc_tile_pool(name="w", bufs=1)
        apool = tc.alloc_tile_pool(name="a", bufs=1)
        pspool = tc.alloc_tile_pool(name="ps", bufs=1, space="PSUM")

        def wtile(ap_, name):
            t = wpool.tile(list(ap_.shape), ap_.dtype, tag=name, name=name)
            nc.sync.dma_start(t[:], ap_[:])
            return t

        tD = wtile(D, "D")
        tcb = wtile(cbias, "cbias")
        tw_b1c1 = wtile(w_b1c1, "w_b1c1")
        tw_b1c2 = wtile(w_b1c2, "w_b1c2")
        tident_r = wtile(ident_r, "ident_r")
        tw_b2c1 = wtile(w_b2c1, "w_b2c1")
        tw_b2sk = wtile(w_b2sk, "w_b2sk")
        tw_b2c2 = wtile(w_b2c2, "w_b2c2")
        tw_b3c1 = wtile(w_b3c1, "w_b3c1")
        tw_b3c2 = wtile(w_b3c2, "w_b3c2")

        def zset(ap_):
            nc.vector.memset(ap_.bitcast(f32) if ap_.dtype == f32r else ap_, 0.0)

        # per-item padded input / front tiles
        xp, mg, h0p = [], [], []
        for par in range(PAR):
            t = apool.tile([128, KC * TP], xp_dt, tag=f"xp{par}", name=f"xp{par}")
            for ch in range(KC):
                zset(t[:, ch * TP:ch * TP + 1])
                zset(t[:, ch * TP + 282:ch * TP + 284])
            zset(t[:, 2 * TP:3 * TP])
            xp.append(t)
            mg.append(apool.tile([128, KC * 128], sdt["front"], tag=f"mg{par}",
                                 name=f"mg{par}"))
            t0 = apool.tile([128, TP], h0_dt, tag=f"h0p{par}", name=f"h0p{par}")
            zset(t0[:, 0:1])
            zset(t0[:, 282:284])
            h0p.append(t0)

        # pair tiles: blocks of TP cols indexed (chunk*2 + sub_item)
        def pairt(name, chunks, dt):
            ts_ = []
            for pp in range(NPP):
                t = apool.tile([128, chunks * 2 * TP], dt, tag=f"{name}{pp}",
                               name=f"{name}{pp}")
                for blk in range(chunks * 2):
                    zset(t[:, blk * TP:blk * TP + 1])
                    zset(t[:, blk * TP + 282:blk * TP + 284])
                ts_.append(t)
            return ts_

        y1p = pairt("y1p", 1, y1_dt)
        h1p = pairt("h1p", 1, h1_dt)
        y2p = pairt("y2p", 2, y2_dt)
        h2p = pairt("h2p", 2, h2_dt)
        y3p = pairt("y3p", 2, y3_dt)
        h3p = pairt("h3p", 2, f32)
        tmp3 = [apool.tile([128, 2 * N], f32, tag=f"tmp3_{i}", name=f"tmp3_{i}")
                for i in range(4)]

        V = [apool.tile([128, n_items], f32, tag=f"V{c}", name=f"V{c}")
             for c in range(2)]

        psum_ctr = [0]

        def pwide():
            t = pspool.tile([128, 1024], f32, tag=f"ps{psum_ctr[0] % 4}",
                            name=f"psum{psum_ctr[0]}")
            psum_ctr[0] += 1
            return t

        def blkwin(tl, blk, k):
            return tl[:, blk * TP + k: blk * TP + k + N]

        def pair_act(p, dst, hblk, bias_col):
            """One ScalarE op covering both items of a pair: PSUM cols
            {0,512}+0..280 -> dst blocks {hblk*2, hblk*2+1} cols 1..281."""
            pin = p[:].rearrange("m (i c) -> m i c", i=2)[:, :, 0:T]
            sout = (dst[:, hblk * 2 * TP:(hblk * 2 + 2) * TP]
                    .rearrange("m (i c) -> m i c", i=2)[:, :, 1:1 + T])
            nc.scalar.activation(sout, pin, AF.Gelu,
                                 bias=tcb[:, bias_col:bias_col + 1])

        def st_load(b, par):
            xt = xp[par]
            for ch in range(KC):
                rows = 128 if ch < 2 else C - 2 * 128
                nc.sync.dma_start(
                    xt[:rows, ch * TP + 1: ch * TP + 1 + T],
                    X[b, ch * 128: ch * 128 + rows, :])
            nc.sync.dma_start(mg[par][:], Mg[b])

        def st_front(b0, pars, pp):
            p = pwide()
            for sub in range(2):
                par = pars[sub]
                for ch in range(KC):
                    nc.tensor.matmul(p[:, sub * 512: sub * 512 + N],
                                     mg[par][:, ch * 128:(ch + 1) * 128],
                                     blkwin(xp[par], ch, 1),
                                     start=(ch == 0), stop=(ch == 2))
            for sub in range(2):
                nc.vector.tensor_scalar_add(
                    h0p[pars[sub]][:, 1:1 + T], p[:, sub * 512: sub * 512 + T],
                    tD[:, b0 + sub:b0 + sub + 1])

        def st_b1c1(b0, pars, pp):
            p = pwide()
            for sub in range(2):
                for k in range(3):
                    nc.tensor.matmul(p[:, sub * 512: sub * 512 + N],
                                     tw_b1c1[:, k * 128:(k + 1) * 128],
                                     blkwin(h0p[pars[sub]], 0, k),
                                     start=(k == 0), stop=(k == 2))
            pair_act(p, y1p[pp], 0, 0)

        def st_b1c2(b0, pars, pp):
            p = pwide()
            for sub in range(2):
                reg = p[:, sub * 512: sub * 512 + N]
                for k in range(3):
                    nc.tensor.matmul(reg, tw_b1c2[:, k * 128:(k + 1) * 128],
                                     blkwin(y1p[pp], sub, k),
                                     start=(k == 0), stop=False)
                nc.tensor.matmul(reg, tident_r[:], blkwin(h0p[pars[sub]], 0, 1),
                                 start=False, stop=True)
            pair_act(p, h1p[pp], 0, 1)

        def st_b2c1(b0, pars, pp, h):
            p = pwide()
            for sub in range(2):
                for k in range(3):
                    nc.tensor.matmul(p[:, sub * 512: sub * 512 + N],
                                     tw_b2c1[:, (k * 2 + h) * 128:(k * 2 + h + 1) * 128],
                                     blkwin(h1p[pp], sub, k),
                                     start=(k == 0), stop=(k == 2))
            pair_act(p, y2p[pp], h, 2 + h)

        def st_b2c2(b0, pars, pp, h):
            p = pwide()
            for sub in range(2):
                reg = p[:, sub * 512: sub * 512 + N]
                first = True
                for k in range(3):
                    for c in range(2):
                        nc.tensor.matmul(
                            reg, tw_b2c2[:, ((k * 2 + c) * 2 + h) * 128:((k * 2 + c) * 2 + h + 1) * 128],
                            blkwin(y2p[pp], c * 2 + sub, k), start=first, stop=False)
                        first = False
                nc.tensor.matmul(reg, tw_b2sk[:, h * 128:(h + 1) * 128],
                                 blkwin(h1p[pp], sub, 1), start=False, stop=True)
            pair_act(p, h2p[pp], h, 4 + h)

        def st_b3c1(b0, pars, pp, h):
            p = pwide()
            for sub in range(2):
                first = True
                for k in range(3):
                    for c in range(2):
                        nc.tensor.matmul(
                            p[:, sub * 512: sub * 512 + N],
                            tw_b3c1[:, ((k * 2 + c) * 2 + h) * 128:((k * 2 + c) * 2 + h + 1) * 128],
                            blkwin(h2p[pp], c * 2 + sub, k), start=first, stop=False)
                        first = False
            pair_act(p, y3p[pp], h, 6 + h)

        def st_b3c2(b0, pars, pp, h):
            p = pwide()
            for sub in range(2):
                first = True
                for k in range(3):
                    for c in range(2):
                        nc.tensor.matmul(
                            p[:, sub * 512: sub * 512 + N],
                            tw_b3c2[:, ((k * 2 + c) * 2 + h) * 128:((k * 2 + c) * 2 + h + 1) * 128],
                            blkwin(y3p[pp], c * 2 + sub, k), start=first, stop=False)
                        first = False
            # residual add on DVE (both items in one op), then pair gelu
            tm = tmp3[(pp % 2) * 2 + h]
            pin = p[:].rearrange("m (i c) -> m i c", i=2)[:, :, 0:N]
            res = (h2p[pp][:, h * 2 * TP:(h * 2 + 2) * TP]
                   .rearrange("m (i c) -> m i c", i=2)[:, :, 1:1 + N])
            tview = tm[:].rearrange("m (i c) -> m i c", i=2)
            nc.vector.tensor_add(tview, pin, res)
            sout = (h3p[pp][:, h * 2 * TP:(h * 2 + 2) * TP]
                    .rearrange("m (i c) -> m i c", i=2)[:, :, 1:1 + T])
            nc.scalar.activation(sout, tview[:, :, 0:T], AF.Gelu,
                                 bias=tcb[:, 8 + h:9 + h])

        def st_pool(b, par, pp, sub):
            for c in range(2):
                nc.vector.reduce_sum(
                    V[c][:, b:b + 1],
                    h3p[pp][:, (c * 2 + sub) * TP:(c * 2 + sub + 1) * TP],
                    axis=AX.X)

        def pair_stages(pp):
            out_ = [st_front, st_b1c1, st_b1c2]
            outl = [lambda b0, pars, pp=pp, f=f: f(b0, pars, pp) for f in out_]
            for f in (st_b2c1, st_b2c2, st_b3c1, st_b3c2):
                for h in range(2):
                    outl.append(lambda b0, pars, pp=pp, f=f, h=h: f(b0, pars, pp, h))
            return outl

        # group of 4 items = 2 pairs, stage-interleaved between the pairs
        for g0 in range(0, n_items, 4):
            bs = list(range(g0, g0 + 4))
            pars = [b % PAR for b in bs]
            pps = [(g0 // 2) % NPP, (g0 // 2 + 1) % NPP]
            for b, par in zip(bs, pars):
                st_load(b, par)
            stA = pair_stages(pps[0])
            stB = pair_stages(pps[1])
            for sA, sB in zip(stA, stB):
                sA(bs[0], pars[0:2])
                sB(bs[2], pars[2:4])
            for i in range(4):
                st_pool(bs[i], pars[i], pps[i // 2], i % 2)

        # ---- head ----
        tw1 = wtile(w1, "w1")
        tw1x = wtile(w1x, "w1x")
        trhsx = wtile(rhsx, "rhsx")
        tw2t = wtile(w2t, "w2t")
        tb2row = wtile(b2row, "b2row")
        tones1 = wtile(ones1, "ones1")

        Vr = [apool.tile([128, n_items], f32r, tag=f"Vr{c}", name=f"Vr{c}")
              for c in range(2)]
        for c in range(2):
            nc.vector.tensor_copy(Vr[c][:], V[c][:])

        ph = pwide()[:, :n_items]
        for c in range(2):
            nc.tensor.matmul(ph[:], tw1[:, c * 128:(c + 1) * 128], Vr[c][:],
                             start=(c == 0), stop=False)
        nc.tensor.matmul(ph[:], tw1x[:], trhsx[:], start=False, stop=True)
        hmid = apool.tile([128, n_items], f32r, tag="hmid", name="hmid")
        nc.scalar.activation(hmid[:], ph[:], AF.Relu)

        out_sb = apool.tile([n_items, NCLS], f32, tag="out_sb", name="out_sb")
        nsplit = [512, 512, 512, NCLS - 3 * 512]
        off = 0
        for w_ in nsplit:
            po = pwide()[:n_items, :w_]
            nc.tensor.matmul(po[:], hmid[:], tw2t[:, off:off + w_],
                             start=True, stop=False)
            nc.tensor.matmul(po[:], tones1[:], tb2row[:, off:off + w_],
                             start=False, stop=True)
            nc.vector.tensor_copy(out_sb[:, off:off + w_], po[:])
            off += w_
        nc.sync.dma_start(out[:, :], out_sb[:, :])

        for p_ in (pspool, apool, wpool):
            p_.release()

    nc.compile()
    return nc


def _preprocess(inputs, sdt):
    f = np.float64

    def npdt(dt):
        return ml_dtypes.bfloat16 if dt == bf16 else np.float32

    attn = inputs["attention"].astype(f)
    attn = attn - attn.max(axis=1, keepdims=True)
    np.exp(attn, out=attn)
    attn /= attn.sum(axis=1, keepdims=True)
    A = inputs["sa_w"].astype(f) @ attn
    subj_w = inputs["subj_w"].astype(f)
    M = np.einsum("shk,kc->shc", subj_w, A)
    MT = np.zeros((NSUBJ, KC * 128, H), np.float32)
    MT[:, :C, :] = np.transpose(M, (0, 2, 1))
    MT = (MT.reshape(NSUBJ, KC, 128, H).transpose(0, 2, 1, 3)
            .reshape(NSUBJ, 128, KC * 128).astype(npdt(sdt["front"])))
    Dall = (np.einsum("shk,k->sh", subj_w, inputs["sa_b"].astype(f))
            + inputs["subj_b"].astype(f)).astype(np.float32)

    inv = 1.0 / np.sqrt(1.0 + EPS)

    def fold(w, b, g, be):
        s = g.astype(f) * inv
        wf = w.astype(f) * s[:, None, None]
        bf_ = s * b.astype(f) + be.astype(f)
        return wf, bf_.astype(np.float32)

    def pack_taps(wf, cin_chunks, cout_halves, dt):
        blocks = []
        for k in range(3):
            for c in range(cin_chunks):
                for h in range(cout_halves):
                    blk = wf[h * 128:(h + 1) * 128, c * 128:(c + 1) * 128, k].T
                    blocks.append(blk)
        return np.concatenate(blocks, axis=1).astype(npdt(dt))

    w11, b11 = fold(inputs["b1_c1w"], inputs["b1_c1b"], inputs["b1_g1"], inputs["b1_be1"])
    w12, b12 = fold(inputs["b1_c2w"], inputs["b1_c2b"], inputs["b1_g2"], inputs["b1_be2"])
    w21, b21 = fold(inputs["b2_c1w"], inputs["b2_c1b"], inputs["b2_g1"], inputs["b2_be1"])
    w22, b22 = fold(inputs["b2_c2w"], inputs["b2_c2b"], inputs["b2_g2"], inputs["b2_be2"])
    w31, b31 = fold(inputs["b3_c1w"], inputs["b3_c1b"], inputs["b3_g1"], inputs["b3_be1"])
    w32, b32 = fold(inputs["b3_c2w"], inputs["b3_c2b"], inputs["b3_g2"], inputs["b3_be2"])
    skw = inputs["b2_skw"][:, :, 0].astype(np.float64)
    skb = inputs["b2_skb"].astype(np.float32)
    b22 = b22 + skb

    cbias = np.zeros((128, 10), np.float32)
    cbias[:, 0] = b11
    cbias[:, 1] = b12
    cbias[:, 2], cbias[:, 3] = b21[:128], b21[128:]
    cbias[:, 4], cbias[:, 5] = b22[:128], b22[128:]
    cbias[:, 6], cbias[:, 7] = b31[:128], b31[128:]
    cbias[:, 8], cbias[:, 9] = b32[:128], b32[128:]

    head_w1 = inputs["head_w1"].astype(f)
    w1pack = np.concatenate(
        [(head_w1[:, c * 128:(c + 1) * 128] / T).T.astype(np.float32) for c in range(2)],
        axis=1)
    w1x = np.zeros((128, 128), np.float32)
    w1x[:E, :] = head_w1[:, 2 * 128:2 * 128 + E].T
    w1x[E, :] = inputs["head_b1"]
    w2t = inputs["head_w2"].T.astype(np.float32)
    b2row = inputs["head_b2"].astype(np.float32)[None, :]

    shared = dict(
        cbias=cbias,
        w_b1c1=pack_taps(w11, 1, 1, sdt["b1c1"]),
        w_b1c2=pack_taps(w12, 1, 1, sdt["b1c2"]),
        w_b2c1=pack_taps(w21, 1, 2, sdt["b2c1"]),
        w_b2sk=np.concatenate([skw[:128].T, skw[128:].T], axis=1).astype(npdt(sdt["b2sk"])),
        w_b2c2=pack_taps(w22, 2, 2, sdt["b2c2"]),
        w_b3c1=pack_taps(w31, 2, 2, sdt["b3c1"]),
        w_b3c2=pack_taps(w32, 2, 2, sdt["b3c2"]),
        ident_r=np.eye(128, dtype=npdt(sdt["b1c2"])),
        w1=w1pack, w1x=w1x, w2t=w2t, b2row=b2row,
    )

    sidx = inputs["subject_idxs"].astype(np.int64)
    Mg = MT[sidx]
    Dcols = Dall[sidx].T.astype(np.float32)
    emb = inputs["emb"].astype(np.float32)
    embG = emb[sidx].T
    return shared, Mg, Dcols, embG


def _run(inputs, n_items, n_cores):
    sdt = _stage_dtypes()
    key = (n_items, n_cores, tuple(sorted((k, str(v)) for k, v in sdt.items())))
    if key not in _CACHE:
        _CACHE[key] = _build(n_items, n_cores, sdt)
    nc = _CACHE[key]

    shared, Mg, Dcols, embG = _preprocess(inputs, sdt)
    xdt = ml_dtypes.bfloat16 if sdt["front"] == bf16 else np.float32
    X = np.ascontiguousarray(inputs["X"], dtype=xdt)

    in_maps = []
    for c in range(n_cores):
        lo, hi = c * n_items, (c + 1) * n_items
        rhsx = np.zeros((128, n_items), np.float32)
        rhsx[:E, :] = embG[:, lo:hi]
        rhsx[E, :] = 1.0
        m = dict(shared)
        m["X"] = X[lo:hi]
        m["Mg"] = np.ascontiguousarray(Mg[lo:hi])
        m["D"] = np.ascontiguousarray(Dcols[:, lo:hi])
        m["rhsx"] = rhsx
        m["ones1"] = np.ones((1, n_items), np.float32)
        in_maps.append(m)

    trace = bool(int(os.environ.get("KTRACE", "0")))
    if trace:
        try:
            from antenv.axon_hooks import (get_axon_ntff_profile_hook,
                                           set_axon_ntff_profile_hook)
            if get_axon_ntff_profile_hook() is None:
                from trn_agent_boot.trn_boot import _ntff_profile_via_ctypes
                set_axon_ntff_profile_hook(
                    _ntff_profile_via_ctypes("/opt/axon/libaxon_pjrt.so"))
        except Exception as e:
            print(f"(ntff hook unavailable: {e})")
    res = run_bass_kernel_spmd(nc, in_maps, core_ids=list(range(n_cores)),
                               trace=trace)
    outp = np.concatenate([res.results[c]["out"] for c in range(n_cores)], axis=0)
    if trace:
        print(f"HW exec time: {res.exec_time_ns} ns "
              f"(mean {res.mean_exec_time_ns}, max core {res.max_exec_time_core_id})")
    return outp, res


def kernel(**inputs):
    outp, _ = _run(inputs, B // 8, 8)
    return outp



# revision 7
# speedup vs baseline: 1.6808x; 1.6808x over previous
"""Trainium2 Bass kernel for BasicConvClassifierWithSubject.

Pure data parallel over 8 cores (128 items/core). All convs run on the PE in
fp8e4m3 using DoubleRow perf mode (two K=128 chunks contracted per
instruction at bf16-instruction cost => 2x throughput). Conv taps pair via
overlapping-window access patterns; 256-channel convs pair their two input
chunks; b1c2's identity residual pairs with tap2 (diag(row_scale) weights).
Activations are stored once, in fp8, in a per-pair-slot "arena" tile so all
pairings are intra-tile with constant strides; residual/skip paths read the
same fp8 copy (validated: final rel err ~1e-4, hardware fp8 rounding is
bit-identical to ml_dtypes). Weights carry per-output-row power-of-2 scales,
descaled for free via the activation scale operand (which is also the fast
path on ScalarE). BN is folded into weights; spatial attention + per-subject
1x1 conv fold into one per-item [271,128] matrix on the host (moe routing as
a host gather). b3c2's residual is added by DVE in-place on PSUM before the
final gelu; global average pooling is DVE column reduces, folded /T into the
head weights. The head runs in f32r.

Items are processed in PAIRS sharing one 2-bank PSUM tile (item a at column
0, item b at 512) so each activation covers both items in one strided
ScalarE instruction. Two pairs interleave (group of 4) so the PE always has
an independent pair's matmuls while ScalarE drains the other.
"""

import os
import numpy as np
import ml_dtypes

import concourse.bass as bass
import concourse.tile as tile
from concourse import bacc, mybir
from concourse.bass_types import AP
from concourse.bass_utils import run_bass_kernel_spmd

f32 = mybir.dt.float32
f32r = mybir.dt.float32r
bf16 = mybir.dt.bfloat16
fp8 = mybir.dt.float8e4
E4 = ml_dtypes.float8_e4m3
AF = mybir.ActivationFunctionType
AX = mybir.AxisListType
ALU = mybir.AluOpType
DRM = mybir.MatmulPerfMode.DoubleRow

B, C, T = 1024, 271, 281
H, H2, E, NCLS, NSUBJ = 128, 256, 16, 1854, 4
EPS = 1e-5
TP = 284          # padded time length per block (zeros at cols 0, 282, 283)
N = 282           # matmul moving size (even)
PAR = 8           # items in flight (2 interleaved pairs x 2 groups deep)
NPP = PAR // 2    # pair slots

# arena block indices (each block TP cols, fp8): see pairing table in _build
BLK_Y1, BLK_H0, BLK_H1, BLK_Y2, BLK_H2, BLK_Y3 = 0, 2, 4, 6, 10, 14
NBLK = 18

_CACHE = {}


def _build(n_items, n_cores):
    assert n_items % 4 == 0
    nc = bacc.Bacc("TRN2", target_bir_lowering=False, debug=False,
                   num_devices=n_cores)

    def din(name, shape, dt=f32):
        return nc.dram_tensor(name, shape, dt, kind="ExternalInput").ap()

    X = din("X", [n_items, C, T], fp8)
    Mg = din("Mg", [n_items, 128, 4 * 128], fp8)
    D = din("D", [128, n_items], f32)            # front bias columns
    SM = din("SM", [128, n_items], f32)          # front descale columns
    cbias = din("cbias", [128, 10], f32)
    cscale = din("cscale", [128, 10], f32)
    w_b1c1 = din("w_b1c1", [128, 3 * 128], fp8)   # (t0,t1) DR + t2 solo
    w_b1c2 = din("w_b1c2", [128, 4 * 128], fp8)   # (t0,t1) + (t2, diag_s)
    w_b2c1 = din("w_b2c1", [128, 6 * 128], fp8)   # per half: (t0,t1) + t2
    w_b2c2 = din("w_b2c2", [128, 14 * 128], fp8)  # per half: 3 DR + skip
    w_b3c1 = din("w_b3c1", [128, 12 * 128], fp8)  # per half: 3 DR
    w_b3c2 = din("w_b3c2", [128, 12 * 128], fp8)
    w1 = din("w1", [128, 2 * 128], f32r)
    w1x = din("w1x", [128, 128], f32r)
    rhsx = din("rhsx", [128, n_items], f32r)
    w2t = din("w2t", [128, NCLS], f32r)
    b2row = din("b2row", [1, NCLS], f32r)
    ones1 = din("ones1", [1, n_items], f32r)
    out = nc.dram_tensor("out", [n_items, NCLS], f32, kind="ExternalOutput").ap()

    with tile.TileContext(nc) as tc:
        wpool = tc.alloc_tile_pool(name="w", bufs=1)
        apool = tc.alloc_tile_pool(name="a", bufs=1)
        pspool = tc.alloc_tile_pool(name="ps", bufs=1, space="PSUM")

        def wtile(ap_, name):
            t = wpool.tile(list(ap_.shape), ap_.dtype, tag=name, name=name)
            nc.sync.dma_start(t[:], ap_[:])
            return t

        tD = wtile(D, "D")
        tSM = wtile(SM, "SM")
        tcb = wtile(cbias, "cbias")
        tcs = wtile(cscale, "cscale")
        tw_b1c1 = wtile(w_b1c1, "w_b1c1")
        tw_b1c2 = wtile(w_b1c2, "w_b1c2")
        tw_b2c1 = wtile(w_b2c1, "w_b2c1")
        tw_b2c2 = wtile(w_b2c2, "w_b2c2")
        tw_b3c1 = wtile(w_b3c1, "w_b3c1")
        tw_b3c2 = wtile(w_b3c2, "w_b3c2")

        # per-item padded input tiles: 4 blocks (chunk 3 stays all-zero)
        xp, mg = [], []
        for par in range(PAR):
            t = apool.tile([128, 4 * TP], fp8, tag=f"xp{par}", name=f"xp{par}")
            nc.vector.memset(t[:, 3 * TP:4 * TP], 0.0)
            nc.vector.memset(t[:, 2 * TP:3 * TP], 0.0)
            for ch in range(2):
                nc.vector.memset(t[:, ch * TP:ch * TP + 1], 0.0)
                nc.vector.memset(t[:, ch * TP + 282:ch * TP + 284], 0.0)
            xp.append(t)
            mg.append(apool.tile([128, 4 * 128], fp8, tag=f"mg{par}",
                                 name=f"mg{par}"))

        # per-pair-slot fp8 activation arena
        arena = []
        for pp in range(NPP):
            t = apool.tile([128, NBLK * TP], fp8, tag=f"ar{pp}", name=f"ar{pp}")
            for blk in range(NBLK):
                nc.vector.memset(t[:, blk * TP:blk * TP + 1], 0.0)
                nc.vector.memset(t[:, blk * TP + 282:blk * TP + 284], 0.0)
            arena.append(t)

        # h3 pair tiles (bf16): blocks c0a c0b c1a c1b
        h3p = []
        for pp in range(NPP):
            t = apool.tile([128, 4 * TP], bf16, tag=f"h3p{pp}", name=f"h3p{pp}")
            for blk in range(4):
                nc.vector.memset(t[:, blk * TP:blk * TP + 1], 0.0)
                nc.vector.memset(t[:, blk * TP + 282:blk * TP + 284], 0.0)
            h3p.append(t)

        V = [apool.tile([128, n_items], f32, tag=f"V{c}", name=f"V{c}")
             for c in range(2)]

        psum_ctr = [0]

        def pwide():
            t = pspool.tile([128, 1024], f32, tag=f"ps{psum_ctr[0] % 4}",
                            name=f"psum{psum_ctr[0]}")
            psum_ctr[0] += 1
            return t

        AR_STRIDE = NBLK * TP

        def drw(tw, pair_idx):
            """DR weight view: [K, 2, M] at 256-col block pair_idx."""
            return tw[:, pair_idx * 256:(pair_idx + 1) * 256].rearrange(
                "k (two m) -> k two m", two=2)

        def ov_rhs(pp, blk, k0, s1):
            """Overlapping/strided DR rhs [K,2,N] in arena pp: slot i at
            column blk*TP + k0 + i*s1."""
            base = arena[pp][:]
            return AP(base.tensor, base.offset + blk * TP + k0,
                      [[AR_STRIDE, 128], [s1, 2], [1, N]],
                      None, base.runtime_checks, None)

        def win(pp, blk, k):
            return arena[pp][:, blk * TP + k: blk * TP + k + N]

        def pair_act(p, pp, blk, col, dst=None):
            """Gelu both items: PSUM cols {0,512}+0..281 -> blocks blk,blk+1
            cols 1..282 (fp8 arena by default)."""
            t = arena[pp] if dst is None else dst
            pin = p[:].rearrange("m (i c) -> m i c", i=2)[:, :, 0:N]
            sout = (t[:, blk * TP:(blk + 2) * TP]
                    .rearrange("m (i c) -> m i c", i=2)[:, :, 1:1 + N])
            nc.scalar.activation(sout, pin, AF.Gelu,
                                 bias=tcb[:, col:col + 1],
                                 scale=tcs[:, col:col + 1])

        def st_load(b, par):
            xt = xp[par]
            for ch in range(3):
                rows = 128 if ch < 2 else C - 2 * 128
                nc.sync.dma_start(
                    xt[:rows, ch * TP + 1: ch * TP + 1 + T],
                    X[b, ch * 128: ch * 128 + rows, :])
            nc.sync.dma_start(mg[par][:], Mg[b])

        def st_front(b0, pars, pp):
            p = pwide()
            for sub in range(2):
                par = pars[sub]
                xb = xp[par][:]
                for half in range(2):
                    rhs = AP(xb.tensor, xb.offset + half * 2 * TP + 1,
                             [[4 * TP, 128], [TP, 2], [1, N]],
                             None, xb.runtime_checks, None)
                    nc.tensor.matmul(p[:, sub * 512: sub * 512 + N],
                                     drw(mg[par], half), rhs,
                                     start=(half == 0), stop=(half == 1),
                                     perf_mode=DRM)
            for sub in range(2):
                hout = arena[pp][:, (BLK_H0 + sub) * TP + 1:
                                 (BLK_H0 + sub) * TP + 1 + N]
                nc.vector.tensor_scalar(
                    hout, p[:, sub * 512: sub * 512 + N],
                    tSM[:, b0 + sub:b0 + sub + 1],
                    tD[:, b0 + sub:b0 + sub + 1],
                    op0=ALU.mult, op1=ALU.add)

        # NOTE: DoubleRow ifmap pair strides must be EVEN (odd strides hang
        # the exec unit) — so taps pair as (t0, t2) at stride 2, never
        # (t0, t1) at stride 1.
        def st_b1c1(b0, pars, pp):
            p = pwide()
            for sub in range(2):
                reg = p[:, sub * 512: sub * 512 + N]
                nc.tensor.matmul(reg, drw(tw_b1c1, 0),
                                 ov_rhs(pp, BLK_H0 + sub, 0, 2),
                                 start=True, stop=False, perf_mode=DRM)
                nc.tensor.matmul(reg, tw_b1c1[:, 256:384],
                                 win(pp, BLK_H0 + sub, 1),
                                 start=False, stop=True)
            pair_act(p, pp, BLK_Y1, 0)

        def st_b1c2(b0, pars, pp):
            p = pwide()
            for sub in range(2):
                reg = p[:, sub * 512: sub * 512 + N]
                nc.tensor.matmul(reg, drw(tw_b1c2, 0),
                                 ov_rhs(pp, BLK_Y1 + sub, 0, 2),
                                 start=True, stop=False, perf_mode=DRM)
                # (tap1 @ y1[k=1], diag_s @ h0[k=1]): stride 2*TP (even)
                nc.tensor.matmul(reg, drw(tw_b1c2, 1),
                                 ov_rhs(pp, BLK_Y1 + sub, 1, 2 * TP),
                                 start=False, stop=True, perf_mode=DRM)
            pair_act(p, pp, BLK_H1, 1)

        def st_b2c1(b0, pars, pp, h):
            p = pwide()
            for sub in range(2):
                reg = p[:, sub * 512: sub * 512 + N]
                nc.tensor.matmul(reg,
                                 tw_b2c1[:, h * 384:h * 384 + 256].rearrange(
                                     "k (two m) -> k two m", two=2),
                                 ov_rhs(pp, BLK_H1 + sub, 0, 2),
                                 start=True, stop=False, perf_mode=DRM)
                nc.tensor.matmul(reg, tw_b2c1[:, h * 384 + 256:h * 384 + 384],
                                 win(pp, BLK_H1 + sub, 1),
                                 start=False, stop=True)
            pair_act(p, pp, BLK_Y2 + 2 * h, 2 + h)

        def st_b2c2(b0, pars, pp, h):
            p = pwide()
            for sub in range(2):
                reg = p[:, sub * 512: sub * 512 + N]
                for k in range(3):
                    nc.tensor.matmul(
                        reg,
                        tw_b2c2[:, h * 896 + k * 256:h * 896 + (k + 1) * 256]
                        .rearrange("k (two m) -> k two m", two=2),
                        ov_rhs(pp, BLK_Y2 + sub, k, 2 * TP),
                        start=(k == 0), stop=False, perf_mode=DRM)
                nc.tensor.matmul(reg,
                                 tw_b2c2[:, h * 896 + 768:h * 896 + 896],
                                 win(pp, BLK_H1 + sub, 1),
                                 start=False, stop=True)
            pair_act(p, pp, BLK_H2 + 2 * h, 4 + h)

        def st_b3c1(b0, pars, pp, h):
            p = pwide()
            for sub in range(2):
                reg = p[:, sub * 512: sub * 512 + N]
                for k in range(3):
                    nc.tensor.matmul(
                        reg,
                        tw_b3c1[:, h * 768 + k * 256:h * 768 + (k + 1) * 256]
                        .rearrange("k (two m) -> k two m", two=2),
                        ov_rhs(pp, BLK_H2 + sub, k, 2 * TP),
                        start=(k == 0), stop=(k == 2), perf_mode=DRM)
            pair_act(p, pp, BLK_Y3 + 2 * h, 6 + h)

        def st_b3c2(b0, pars, pp, h):
            p = pwide()
            for sub in range(2):
                reg = p[:, sub * 512: sub * 512 + N]
                for k in range(3):
                    nc.tensor.matmul(
                        reg,
                        tw_b3c2[:, h * 768 + k * 256:h * 768 + (k + 1) * 256]
                        .rearrange("k (two m) -> k two m", two=2),
                        ov_rhs(pp, BLK_Y3 + sub, k, 2 * TP),
                        start=(k == 0), stop=(k == 2), perf_mode=DRM)
            # residual: psum <- psum * (1/s_row) + h2 (in place), then gelu
            pin = p[:].rearrange("m (i c) -> m i c", i=2)[:, :, 0:N]
            res = (arena[pp][:, (BLK_H2 + 2 * h) * TP:(BLK_H2 + 2 * h + 2) * TP]
                   .rearrange("m (i c) -> m i c", i=2)[:, :, 1:1 + N])
            nc.vector.scalar_tensor_tensor(
                pin, pin, tcs[:, 8 + h:9 + h], res, op0=ALU.mult, op1=ALU.add)
            sout = (h3p[pp][:, 2 * h * TP:(2 * h + 2) * TP]
                    .rearrange("m (i c) -> m i c", i=2)[:, :, 1:1 + N])
            nc.scalar.activation(sout, pin, AF.Gelu, bias=tcb[:, 8 + h:9 + h])

        def st_pool(b, par, pp, sub):
            for c in range(2):
                nc.vector.reduce_sum(
                    V[c][:, b:b + 1],
                    h3p[pp][:, (c * 2 + sub) * TP:(c * 2 + sub + 1) * TP],
                    axis=AX.X)

        def pair_stages(pp):
            outl = [lambda b0, pars, pp=pp, f=f: f(b0, pars, pp)
                    for f in (st_front, st_b1c1, st_b1c2)]
            for f in (st_b2c1, st_b2c2, st_b3c1, st_b3c2):
                for h in range(2):
                    outl.append(lambda b0, pars, pp=pp, f=f, h=h: f(b0, pars, pp, h))
            return outl

        for g0 in range(0, n_items, 4):
            bs = list(range(g0, g0 + 4))
            pars = [b % PAR for b in bs]
            pps = [(g0 // 2) % NPP, (g0 // 2 + 1) % NPP]
            for b, par in zip(bs, pars):
                st_load(b, par)
            stA = pair_stages(pps[0])
            stB = pair_stages(pps[1])
            for sA, sB in zip(stA, stB):
                sA(bs[0], pars[0:2])
                sB(bs[2], pars[2:4])
            for i in range(4):
                st_pool(bs[i], pars[i], pps[i // 2], i % 2)

        # ---- head ----
        tw1 = wtile(w1, "w1")
        tw1x = wtile(w1x, "w1x")
        trhsx = wtile(rhsx, "rhsx")
        tw2t = wtile(w2t, "w2t")
        tb2row = wtile(b2row, "b2row")
        tones1 = wtile(ones1, "ones1")

        Vr = [apool.tile([128, n_items], f32r, tag=f"Vr{c}", name=f"Vr{c}")
              for c in range(2)]
        for c in range(2):
            nc.vector.tensor_copy(Vr[c][:], V[c][:])

        ph = pwide()[:, :n_items]
        for c in range(2):
            nc.tensor.matmul(ph[:], tw1[:, c * 128:(c + 1) * 128], Vr[c][:],
                             start=(c == 0), stop=False)
        nc.tensor.matmul(ph[:], tw1x[:], trhsx[:], start=False, stop=True)
        hmid = apool.tile([128, n_items], f32r, tag="hmid", name="hmid")
        nc.scalar.activation(hmid[:], ph[:], AF.Relu)

        out_sb = apool.tile([n_items, NCLS], f32, tag="out_sb", name="out_sb")
        nsplit = [512, 512, 512, NCLS - 3 * 512]
        off = 0
        for w_ in nsplit:
            po = pwide()[:n_items, :w_]
            nc.tensor.matmul(po[:], hmid[:], tw2t[:, off:off + w_],
                             start=True, stop=False)
            nc.tensor.matmul(po[:], tones1[:], tb2row[:, off:off + w_],
                             start=False, stop=True)
            nc.vector.tensor_copy(out_sb[:, off:off + w_], po[:])
            off += w_
        nc.sync.dma_start(out[:, :], out_sb[:, :])

        for p_ in (pspool, apool, wpool):
            p_.release()

    nc.compile()
    return nc


def _pow2_rowscale(w, cap=None):
    """Per-row power-of-2 scale so max|w*s| ~ 224. w: [rows, ...]."""
    mx = np.abs(w).max(axis=tuple(range(1, w.ndim))) + 1e-30
    e = np.floor(np.log2(224.0 / mx))
    if cap is not None:
        e = np.minimum(e, cap)
    return 2.0 ** e


def _preprocess(inputs):
    f = np.float64

    attn = inputs["attention"].astype(f)
    attn = attn - attn.max(axis=1, keepdims=True)
    np.exp(attn, out=attn)
    attn /= attn.sum(axis=1, keepdims=True)
    A = inputs["sa_w"].astype(f) @ attn
    subj_w = inputs["subj_w"].astype(f)
    M = np.einsum("shk,kc->shc", subj_w, A)          # [S,H,C]
    # per-(subject,row) scale
    sM = np.stack([_pow2_rowscale(M[s]) for s in range(NSUBJ)])  # [S,H]
    Ms = M * sM[:, :, None]
    MT = np.zeros((NSUBJ, 4 * 128, H), np.float32)
    MT[:, :C, :] = np.transpose(Ms, (0, 2, 1))
    MT = (MT.reshape(NSUBJ, 4, 128, H).transpose(0, 2, 1, 3)
            .reshape(NSUBJ, 128, 4 * 128).astype(E4))
    Dall = (np.einsum("shk,k->sh", subj_w, inputs["sa_b"].astype(f))
            + inputs["subj_b"].astype(f)).astype(np.float32)

    inv = 1.0 / np.sqrt(1.0 + EPS)

    def fold(p, g, be):
        w = inputs[p + "w"].astype(f)
        s = inputs[g].astype(f) * inv
        bf_ = s * inputs[p + "b"].astype(f) + inputs[be].astype(f)
        return w * s[:, None, None], bf_.astype(np.float32)

    w11, b11 = fold("b1_c1", "b1_g1", "b1_be1")
    w12, b12 = fold("b1_c2", "b1_g2", "b1_be2")
    w21, b21 = fold("b2_c1", "b2_g1", "b2_be1")
    w22, b22 = fold("b2_c2", "b2_g2", "b2_be2")
    w31, b31 = fold("b3_c1", "b3_g1", "b3_be1")
    w32, b32 = fold("b3_c2", "b3_g2", "b3_be2")
    skw = inputs["b2_skw"][:, :, 0].astype(f)
    b22 = b22 + inputs["b2_skb"].astype(np.float32)

    s11 = _pow2_rowscale(w11)
    s12 = _pow2_rowscale(w12, cap=7.0)   # diag entries must stay <= 240 in fp8
    s21 = _pow2_rowscale(w21)
    s22 = _pow2_rowscale(np.concatenate([w22.reshape(H2, -1), skw], axis=1))
    s31 = _pow2_rowscale(w31)
    s32 = _pow2_rowscale(w32)

    def tap(w, s, k, cin_lo, cout_lo):
        """[128,128] fp8 block: (s_row * w[cout_lo:+128, cin_lo:+128, k]).T"""
        blk = (w[cout_lo:cout_lo + 128, cin_lo:cin_lo + 128, k]
               * s[cout_lo:cout_lo + 128, None])
        return blk.T.astype(E4)

    # b1c1: [t0|t2] DR + t1 solo (even-stride tap pairing)
    w_b1c1 = np.concatenate(
        [tap(w11, s11, 0, 0, 0), tap(w11, s11, 2, 0, 0),
         tap(w11, s11, 1, 0, 0)], axis=1)
    # b1c2: [t0|t2] + [t1|diag(s12)]
    diag12 = np.zeros((128, 128), f)
    diag12[np.arange(128), np.arange(128)] = s12
    w_b1c2 = np.concatenate(
        [tap(w12, s12, 0, 0, 0), tap(w12, s12, 2, 0, 0),
         tap(w12, s12, 1, 0, 0), diag12.T.astype(E4)], axis=1)
    # b2c1 per half: [t0|t2] + t1 solo
    w_b2c1 = np.concatenate(
        [blk for h in range(2)
         for blk in (tap(w21, s21, 0, 0, h * 128), tap(w21, s21, 2, 0, h * 128),
                     tap(w21, s21, 1, 0, h * 128))], axis=1)
    # b2c2 per half: [t0c0|t0c1][t1c0|t1c1][t2c0|t2c1] + skip
    def half22(h):
        blks = []
        for k in range(3):
            for cch in range(2):
                blks.append(tap(w22, s22, k, cch * 128, h * 128))
        sk = (skw[h * 128:(h + 1) * 128, :] * s22[h * 128:(h + 1) * 128, None])
        blks.append(sk.T.astype(E4))
        return blks
    w_b2c2 = np.concatenate(half22(0) + half22(1), axis=1)

    def pack3(w, s):
        blks = []
        for h in range(2):
            for k in range(3):
                for cch in range(2):
                    blks.append(tap(w, s, k, cch * 128, h * 128))
        return np.concatenate(blks, axis=1)
    w_b3c1 = pack3(w31, s31)
    w_b3c2 = pack3(w32, s32)

    cbias = np.zeros((128, 10), np.float32)
    cscale = np.zeros((128, 10), np.float32)
    for col, (bvec, svec) in enumerate((
            (b11, s11), (b12, s12),
            (b21[:128], s21[:128]), (b21[128:], s21[128:]),
            (b22[:128], s22[:128]), (b22[128:], s22[128:]),
            (b31[:128], s31[:128]), (b31[128:], s31[128:]),
            (b32[:128], s32[:128]), (b32[128:], s32[128:]))):
        cbias[:, col] = bvec
        cscale[:, col] = 1.0 / svec

    head_w1 = inputs["head_w1"].astype(f)
    w1pack = np.concatenate(
        [(head_w1[:, c * 128:(c + 1) * 128] / T).T.astype(np.float32)
         for c in range(2)], axis=1)
    w1x = np.zeros((128, 128), np.float32)
    w1x[:E, :] = head_w1[:, 2 * 128:2 * 128 + E].T
    w1x[E, :] = inputs["head_b1"]
    w2t = inputs["head_w2"].T.astype(np.float32)
    b2r = inputs["head_b2"].astype(np.float32)[None, :]

    shared = dict(
        cbias=cbias, cscale=cscale,
        w_b1c1=w_b1c1, w_b1c2=w_b1c2, w_b2c1=w_b2c1, w_b2c2=w_b2c2,
        w_b3c1=w_b3c1, w_b3c2=w_b3c2,
        w1=w1pack, w1x=w1x, w2t=w2t, b2row=b2r,
    )

    sidx = inputs["subject_idxs"].astype(np.int64)
    Mg = MT[sidx]
    Dcols = Dall[sidx].T.astype(np.float32)
    SMcols = (1.0 / sM)[sidx].T.astype(np.float32)
    embG = inputs["emb"].astype(np.float32)[sidx].T
    return shared, Mg, Dcols, SMcols, embG


def _run(inputs, n_items, n_cores):
    key = (n_items, n_cores)
    if key not in _CACHE:
        _CACHE[key] = _build(n_items, n_cores)
    nc = _CACHE[key]

    shared, Mg, Dcols, SMcols, embG = _preprocess(inputs)
    X = np.ascontiguousarray(inputs["X"]).astype(E4)

    in_maps = []
    for c in range(n_cores):
        lo, hi = c * n_items, (c + 1) * n_items
        rhsx = np.zeros((128, n_items), np.float32)
        rhsx[:E, :] = embG[:, lo:hi]
        rhsx[E, :] = 1.0
        m = dict(shared)
        m["X"] = X[lo:hi]
        m["Mg"] = np.ascontiguousarray(Mg[lo:hi])
        m["D"] = np.ascontiguousarray(Dcols[:, lo:hi])
        m["SM"] = np.ascontiguousarray(SMcols[:, lo:hi])
        m["rhsx"] = rhsx
        m["ones1"] = np.ones((1, n_items), np.float32)
        in_maps.append(m)

    trace = bool(int(os.environ.get("KTRACE", "0")))
    if trace:
        try:
            from antenv.axon_hooks import (get_axon_ntff_profile_hook,
                                           set_axon_ntff_profile_hook)
            if get_axon_ntff_profile_hook() is None:
                from trn_agent_boot.trn_boot import _ntff_profile_via_ctypes
                set_axon_ntff_profile_hook(
                    _ntff_profile_via_ctypes("/opt/axon/libaxon_pjrt.so"))
        except Exception as e:
            print(f"(ntff hook unavailable: {e})")
    res = run_bass_kernel_spmd(nc, in_maps, core_ids=list(range(n_cores)),
                               trace=trace)
    outp = np.concatenate([res.results[c]["out"] for c in range(n_cores)], axis=0)
    if trace:
        print(f"HW exec time: {res.exec_time_ns} ns "
              f"(mean {res.mean_exec_time_ns}, max core {res.max_exec_time_core_id})")
    return outp, res


def kernel(**inputs):
    outp, _ = _run(inputs, B // 8, 8)
    return outp
